# revision 2
# baseline (speedup 1.0000x reference)
"""Trainium2 Bass kernel for nn_ContrastLoss (LayerNorm + label segment-sum +
EMA codebook contrast loss), data-parallel over 8 NeuronCores.

Contract: kernel(**inputs) takes the FULL unsharded inputs
  input_f [128,1024,768] f32, char_dic [96,768] f32, ln_w [768] f32,
  ln_b [768] f32, target [128,1024] int64
and returns the full output (f32 scalar), matching reference.reference.

Strategy (hardcoded for the shapes above):
 - shard batch dim over 8 cores: 16 batches = 16384 tokens per core
 - per core, stream 32 tiles of [128 partitions x 4 tokens x 768]:
     bn_stats/bn_aggr (DVE) -> per-token mean/var
     rstd via Sqrt(ACT) + reciprocal(DVE)
     normalized tokens cast to bf16 via ACT activation (scale/bias per token)
     one-hot(label) built on GPSIMD via is_equal against an iota constant
     TensorE matmuls accumulate segment sums [96,768] + counts [96,1] in PSUM
   (ln_w/ln_b are folded out of the streaming loop: segment_sum(LN_nw(x)),
    then tok_sums = S*w + counts*b applied once at the end)
 - AllReduce the [96,769] partial sums+counts across the 8 cores
 - tail math (group sums, positive term, EMA update, LayerNorm, negative
   term) computed replicated on every core; host reads core 0's scalar
"""

import os
import sys

for _p in ("/opt/trn_rl_repo",):
    if _p not in sys.path:
        sys.path.insert(0, _p)

import numpy as np

import concourse.bass as bass
import concourse.bacc as bacc
import concourse.tile as tile
from concourse import mybir
from concourse.bass_utils import run_bass_kernel_spmd

F32 = mybir.dt.float32
BF16 = mybir.dt.bfloat16
AF = mybir.ActivationFunctionType
OP = mybir.AluOpType

N_CORES = 8
B, S, D = 128, 1024, 768
NCHAR = 96
EPS = 1e-5
EMA = 0.1

TOK_PER_CORE = (B // N_CORES) * S          # 16384
TOK_PER_PART = 4                           # tokens per partition per tile
TILE_TOK = 128 * TOK_PER_PART              # 512 tokens per tile
N_TILES = TOK_PER_CORE // TILE_TOK         # 32


def build_kernel():
    nc = bacc.Bacc("TRN2", target_bir_lowering=False, debug=False,
                   num_devices=N_CORES)

    x_d = nc.dram_tensor("x", [TOK_PER_CORE, D], F32, kind="ExternalInput")
    lab_d = nc.dram_tensor("lab", [TOK_PER_CORE], F32, kind="ExternalInput")
    char_d = nc.dram_tensor("char", [NCHAR, D], F32, kind="ExternalInput")
    wbc_d = nc.dram_tensor("wbc", [NCHAR, D], F32, kind="ExternalInput")
    bbc_d = nc.dram_tensor("bbc", [NCHAR, D], F32, kind="ExternalInput")
    out_d = nc.dram_tensor("out", [1, 1], F32, kind="ExternalOutput")

    # constants embedded in the NEFF
    iota_np = np.tile(np.arange(NCHAR, dtype=np.float32), (128, 1))
    iota_d = nc.inline_tensor(iota_np, name="iota96")
    mask_np = np.ones((NCHAR, 1), dtype=np.float32)
    mask_np[0, 0] = 0.0
    mask_d = nc.inline_tensor(mask_np, name="maskrow")
    ones96_d = nc.inline_tensor(np.ones((NCHAR, 1), dtype=np.float32),
                                name="ones96")

    # collective bounce buffers
    cc_in = nc.dram_tensor("cc_in", [NCHAR, D + 1], F32)
    cc_out = nc.dram_tensor("cc_out", [NCHAR, D + 1], F32, addr_space="Shared")

    x_r = x_d.ap().rearrange("(t p f) d -> t p f d",
                             t=N_TILES, p=128, f=TOK_PER_PART)
    lab_r = lab_d.ap().rearrange("(t p f) -> t p f",
                                 t=N_TILES, p=128, f=TOK_PER_PART)

    with tile.TileContext(nc) as tc:
        with (
            tc.tile_pool(name="consts", bufs=1) as consts,
            tc.tile_pool(name="xp", bufs=3) as xp,
            tc.tile_pool(name="lp", bufs=3) as lp,
            tc.tile_pool(name="stp", bufs=3) as stp,
            tc.tile_pool(name="ohp", bufs=3) as ohp,
            tc.tile_pool(name="xcp", bufs=3) as xcp,
            tc.tile_pool(name="tailp", bufs=1) as tailp,
            tc.tile_pool(name="psum", bufs=1, space="PSUM") as psp,
        ):
            # --- constants into SBUF ---
            iota_sb = consts.tile([128, NCHAR], F32)
            nc.sync.dma_start(out=iota_sb[:], in_=iota_d.ap())
            ones_bf = consts.tile([128, 1], BF16)
            nc.vector.memset(ones_bf[:], 1.0)
            eps128 = consts.tile([128, 1], F32)
            nc.vector.memset(eps128[:], EPS)
            eps96 = consts.tile([NCHAR, 1], F32)
            nc.vector.memset(eps96[:], EPS)
            mask_sb = consts.tile([NCHAR, 1], F32)
            nc.sync.dma_start(out=mask_sb[:], in_=mask_d.ap())
            ones96_sb = consts.tile([NCHAR, 1], F32)
            nc.sync.dma_start(out=ones96_sb[:], in_=ones96_d.ap())
            char_sb = consts.tile([NCHAR, D], F32)
            nc.sync.dma_start(out=char_sb[:], in_=char_d.ap())
            wbc_sb = consts.tile([NCHAR, D], F32)
            nc.sync.dma_start(out=wbc_sb[:], in_=wbc_d.ap())
            bbc_sb = consts.tile([NCHAR, D], F32)
            nc.sync.dma_start(out=bbc_sb[:], in_=bbc_d.ap())

            # --- PSUM accumulators for the streaming segment-sum ---
            psA = psp.tile([NCHAR, 384], F32)
            psB = psp.tile([NCHAR, 384], F32)
            psC = psp.tile([NCHAR, 1], F32)

            # --- streaming loop ---
            for i in range(N_TILES):
                x_t = xp.tile([128, TOK_PER_PART, D], F32)
                nc.sync.dma_start(out=x_t[:], in_=x_r[i])
                l_t = lp.tile([128, TOK_PER_PART], F32)
                nc.sync.dma_start(out=l_t[:], in_=lab_r[i])

                # per-token stats: 768 = 2 groups x 384
                bn_t = stp.tile([128, 2 * TOK_PER_PART, 6], F32)
                x_g = x_t[:].rearrange("p f (g s) -> p (f g) s", g=2)
                for g in range(2 * TOK_PER_PART):
                    nc.vector.bn_stats(bn_t[:, g, :], x_g[:, g, :])
                st_t = stp.tile([128, TOK_PER_PART, 2], F32)
                for t in range(TOK_PER_PART):
                    nc.vector.bn_aggr(st_t[:, t, :], bn_t[:, 2 * t:2 * t + 2, :])
                mean_v = st_t[:].rearrange("p f two -> p (f two)")[:, 0::2]
                var_v = st_t[:].rearrange("p f two -> p (f two)")[:, 1::2]

                std_t = stp.tile([128, TOK_PER_PART], F32)
                nc.scalar.activation(std_t[:], var_v, AF.Sqrt,
                                     bias=eps128[:], scale=1.0)
                rstd_t = stp.tile([128, TOK_PER_PART], F32)
                nc.vector.reciprocal(rstd_t[:], std_t[:])
                # negmur = -(mean * rstd)
                nmr_t = stp.tile([128, TOK_PER_PART], F32)
                nc.vector.scalar_tensor_tensor(nmr_t[:], mean_v, -1.0,
                                               rstd_t[:], OP.mult, OP.mult)

                # one-hot labels (bf16 0/1), per token slot
                oh_t = ohp.tile([128, TOK_PER_PART, NCHAR], BF16)
                for t in range(TOK_PER_PART):
                    nc.gpsimd.tensor_scalar(oh_t[:, t, :], iota_sb[:],
                                            l_t[:, t:t + 1], None, OP.is_equal)

                # normalized tokens (no w/b), cast to bf16
                xc_t = xcp.tile([128, TOK_PER_PART, D], BF16)
                for t in range(TOK_PER_PART):
                    nc.scalar.activation(xc_t[:, t, :], x_t[:, t, :],
                                         AF.Identity,
                                         bias=nmr_t[:, t:t + 1],
                                         scale=rstd_t[:, t:t + 1])

                first, last = i == 0, i == N_TILES - 1
                for t in range(TOK_PER_PART):
                    st0 = first and t == 0
                    sp0 = last and t == TOK_PER_PART - 1
                    nc.tensor.matmul(psA[:], oh_t[:, t, :],
                                     xc_t[:, t, 0:384], start=st0, stop=sp0)
                    nc.tensor.matmul(psB[:], oh_t[:, t, :],
                                     xc_t[:, t, 384:768], start=st0, stop=sp0)
                    nc.tensor.matmul(psC[:], oh_t[:, t, :],
                                     ones_bf[:], start=st0, stop=sp0)

            # --- local partials -> DRAM -> AllReduce ---
            acc = tailp.tile([NCHAR, D + 1], F32)
            nc.vector.tensor_copy(acc[:, 0:384], psA[:])
            nc.vector.tensor_copy(acc[:, 384:768], psB[:])
            nc.vector.tensor_copy(acc[:, 768:769], psC[:])
            nc.sync.dma_start(out=cc_in.ap(), in_=acc[:])
            nc.gpsimd.collective_compute(
                "AllReduce", OP.add,
                replica_groups=[list(range(N_CORES))],
                ins=[cc_in.ap()], outs=[cc_out.ap()],
            )
            red = tailp.tile([NCHAR, D + 1], F32)
            nc.sync.dma_start(out=red[:], in_=cc_out.ap())
            counts = red[:, 768:769]

            # group_sum = char + S*w + counts*b
            tmp1 = tailp.tile([NCHAR, D], F32)
            nc.vector.scalar_tensor_tensor(tmp1[:], bbc_sb[:], counts,
                                           char_sb[:], OP.mult, OP.add)
            group = tailp.tile([NCHAR, D], F32)
            nc.vector.tensor_mul(group[:], red[:, 0:768], wbc_sb[:])
            nc.vector.tensor_add(group[:], group[:], tmp1[:])

            # positive = sum(group^2) (divide by D at the very end)
            sq = tailp.tile([NCHAR, D], F32)
            pos_col = tailp.tile([NCHAR, 1], F32)
            nc.scalar.activation(sq[:], group[:], AF.Square,
                                 accum_out=pos_col[:])
            pos_ps = psp.tile([1, 1], F32)
            nc.tensor.matmul(pos_ps[:], ones96_sb[:], pos_col[:],
                             start=True, stop=True)
            pos_sb = tailp.tile([1, 1], F32)
            nc.vector.tensor_copy(pos_sb[:], pos_ps[:])

            # EMA update: new_char = char + 0.1 * group/(counts+1); row 0 kept
            cnt1 = tailp.tile([NCHAR, 1], F32)
            nc.vector.tensor_scalar(cnt1[:], counts, 1.0, None, OP.add)
            invc = tailp.tile([NCHAR, 1], F32)
            nc.vector.reciprocal(invc[:], cnt1[:])
            ema = tailp.tile([NCHAR, D], F32)
            nc.vector.tensor_scalar(ema[:], group[:], invc[:], EMA,
                                    OP.mult, OP.mult)
            newc = tailp.tile([NCHAR, D], F32)
            nc.vector.tensor_add(newc[:], char_sb[:], ema[:])
            nc.vector.tensor_copy(newc[0:1, :], char_sb[0:1, :])

            # LayerNorm(new_char) with w/b
            bn2 = tailp.tile([NCHAR, 2, 6], F32)
            for g in range(2):
                nc.vector.bn_stats(bn2[:, g, :], newc[:, g * 384:(g + 1) * 384])
            st2 = tailp.tile([NCHAR, 2], F32)
            nc.vector.bn_aggr(st2[:], bn2[:])
            std2 = tailp.tile([NCHAR, 1], F32)
            nc.scalar.activation(std2[:], st2[:, 1:2], AF.Sqrt,
                                 bias=eps96[:], scale=1.0)
            rstd2 = tailp.tile([NCHAR, 1], F32)
            nc.vector.reciprocal(rstd2[:], std2[:])
            nmr2 = tailp.tile([NCHAR, 1], F32)
            nc.vector.scalar_tensor_tensor(nmr2[:], st2[:, 0:1], -1.0,
                                           rstd2[:], OP.mult, OP.mult)
            nrm = tailp.tile([NCHAR, D], F32)
            nc.scalar.activation(nrm[:], newc[:], AF.Identity,
                                 bias=nmr2[:], scale=rstd2[:])
            fin = tailp.tile([NCHAR, D], F32)
            nc.vector.tensor_mul(fin[:], nrm[:], wbc_sb[:])
            nc.vector.tensor_add(fin[:], fin[:], bbc_sb[:])

            # s = sum over rows 1..95 -> [1,768]; negative = sum(s^2)
            sA = psp.tile([1, 384], F32)
            sB = psp.tile([1, 384], F32)
            nc.tensor.matmul(sA[:], mask_sb[:], fin[:, 0:384],
                             start=True, stop=True)
            nc.tensor.matmul(sB[:], mask_sb[:], fin[:, 384:768],
                             start=True, stop=True)
            sqA = tailp.tile([1, 384], F32)
            sqB = tailp.tile([1, 384], F32)
            negA = tailp.tile([1, 1], F32)
            negB = tailp.tile([1, 1], F32)
            nc.scalar.activation(sqA[:], sA[:], AF.Square, accum_out=negA[:])
            nc.scalar.activation(sqB[:], sB[:], AF.Square, accum_out=negB[:])

            res = tailp.tile([1, 1], F32)
            nc.vector.tensor_add(res[:], negA[:], negB[:])
            nc.vector.tensor_sub(res[:], res[:], pos_sb[:])
            nc.vector.tensor_scalar(res[:], res[:], 1.0 / D, None, OP.mult)
            nc.sync.dma_start(out=out_d.ap(), in_=res[:])

    nc.finalize()
    return nc


_NC_CACHE = None


def _get_nc():
    global _NC_CACHE
    if _NC_CACHE is None:
        _NC_CACHE = build_kernel()
    return _NC_CACHE


def make_in_maps(input_f, char_dic, ln_w, ln_b, target):
    input_f = np.ascontiguousarray(np.asarray(input_f, dtype=np.float32))
    char_dic = np.ascontiguousarray(np.asarray(char_dic, dtype=np.float32))
    ln_w = np.asarray(ln_w, dtype=np.float32)
    ln_b = np.asarray(ln_b, dtype=np.float32)
    labels = np.asarray(target).reshape(B, S).astype(np.float32)

    wbc = np.ascontiguousarray(np.broadcast_to(ln_w[None, :], (NCHAR, D)))
    bbc = np.ascontiguousarray(np.broadcast_to(ln_b[None, :], (NCHAR, D)))

    bpc = B // N_CORES
    in_maps = []
    for c in range(N_CORES):
        x_c = input_f[c * bpc:(c + 1) * bpc].reshape(TOK_PER_CORE, D)
        l_c = labels[c * bpc:(c + 1) * bpc].reshape(TOK_PER_CORE)
        in_maps.append({
            "x": np.ascontiguousarray(x_c),
            "lab": np.ascontiguousarray(l_c),
            "char": char_dic,
            "wbc": wbc,
            "bbc": bbc,
        })
    return in_maps


def run(trace=False, **inputs):
    nc = _get_nc()
    in_maps = make_in_maps(**inputs)
    res = run_bass_kernel_spmd(nc, in_maps, core_ids=list(range(N_CORES)),
                               trace=trace)
    out = np.float32(res.results[0]["out"][0, 0])
    return out, res


def kernel(**inputs):
    out, _ = run(trace=False, **inputs)
    return np.array(out, dtype=np.float32)


if __name__ == "__main__":
    import json
    np.random.seed(0)
    input_f = np.random.randn(B, S, D).astype(np.float32)
    char_dic = np.random.randn(NCHAR, D).astype(np.float32)
    ln_w = np.ones(D, np.float32)
    ln_b = np.zeros(D, np.float32)
    target = np.random.randint(0, NCHAR, (B, S)).astype(np.int64)
    out = kernel(input_f=input_f, char_dic=char_dic, ln_w=ln_w,
                 ln_b=ln_b, target=target)
    print("kernel out:", out)
